# revision 20
# baseline (speedup 1.0000x reference)
"""Bahdanau attention Trainium2 kernel.

Problem: B=32, S=8192, HE=HD=256.
  q_proj = query @ Wa^T + Wa_b                  [B,1,256]
  k_proj = keys @ Ua^T + Ua_b                   [B,S,256]
  scores = tanh(q_proj + k_proj) @ Va + Va_b    [B,S]   (Va_b dropped: softmax shift-invariant)
  weights = softmax(scores)                     [B,1,S]
  context = weights @ keys                      [B,1,256]

Sharding: pure data parallel over batch, 4 batches per core x 8 cores.
Per core, per batch: keys [8192,256] stays resident in SBUF (8MB); streamed in
512-row blocks through PE-transpose -> Ua matmul (float32r) -> tanh(+qbias) on
ACT -> Va matvec producing scores in column layout [128,64]; softmax with
cross-partition reductions done via tiny PE matmuls; context accumulated over
the resident natural-layout keys tiles.
"""

import sys
import numpy as np

for p in ("/opt/trn_rl_repo", "/root/.axon_site/_ro/trn_rl_repo"):
    if p not in sys.path:
        sys.path.append(p)

B, S, HE, HD = 32, 8192, 256, 256
NCORES = 8
BPC = B // NCORES          # batches per core = 4
SB = 512                   # s-block streamed per iteration
NBLK = S // SB             # 16 blocks per batch
NTAU = S // 128            # 64 s-subtiles of 128 per batch

_PROGRAM = None
PROFILE = False          # set by test.py to collect an NTFF trace
LAST_RESULT = None


def _build_program():
    import concourse.bass as bass
    import concourse.bacc as bacc
    import concourse.mybir as mybir
    from concourse import tile
    from contextlib import ExitStack

    f32 = mybir.dt.float32
    f32r = mybir.dt.float32r
    AF = mybir.ActivationFunctionType
    AX = mybir.AxisListType

    nc = bacc.Bacc(trn_type="TRN2")

    keys_d = nc.dram_tensor("keys", [BPC, S, HE], f32r, kind="ExternalInput")
    qT_d = nc.dram_tensor("qT", [128, 2 * BPC], f32, kind="ExternalInput")
    waT_d = nc.dram_tensor("waT", [128, 512], f32, kind="ExternalInput")
    uaT_d = nc.dram_tensor("uaT", [128, 512], f32r, kind="ExternalInput")
    vaT_d = nc.dram_tensor("vaT", [128, 2], f32, kind="ExternalInput")
    abias_d = nc.dram_tensor("abias", [128, 2 * BPC], f32, kind="ExternalInput")
    ident_d = nc.dram_tensor("ident", [128, 128], f32, kind="ExternalInput")
    bcneg_d = nc.dram_tensor("bcneg", [1, 128], f32, kind="ExternalInput")
    bcpos_d = nc.dram_tensor("bcpos", [1, 128], f32, kind="ExternalInput")
    colones_d = nc.dram_tensor("colones", [128, 1], f32, kind="ExternalInput")

    octx_d = nc.dram_tensor("octx", [BPC, HE], f32, kind="ExternalOutput")
    ow_d = nc.dram_tensor("ow", [BPC, S], f32, kind="ExternalOutput")

    with tile.TileContext(nc) as tc, ExitStack() as ctx:
        cpool = ctx.enter_context(tc.tile_pool(name="consts", bufs=1))
        kres = ctx.enter_context(tc.tile_pool(name="kres", bufs=2 * NBLK))
        ktp = ctx.enter_context(tc.tile_pool(name="ktp", bufs=4))
        thp = ctx.enter_context(tc.tile_pool(name="thp", bufs=4))
        smp = ctx.enter_context(tc.tile_pool(name="smp", bufs=2))
        wcp = ctx.enter_context(tc.tile_pool(name="wcp", bufs=2))
        orow = ctx.enter_context(tc.tile_pool(name="orow", bufs=2))

        ptp = ctx.enter_context(tc.tile_pool(name="ptp", bufs=2, space="PSUM"))
        pkp = ctx.enter_context(tc.tile_pool(name="pkp", bufs=2, space="PSUM"))
        pscT = ctx.enter_context(tc.tile_pool(name="pscT", bufs=2, space="PSUM"))
        pctx = ctx.enter_context(tc.tile_pool(name="pctx", bufs=1, space="PSUM"))
        pmisc = ctx.enter_context(tc.tile_pool(name="pmisc", bufs=1, space="PSUM"))

        def ld(pool, dram, shape, tag, dt=None):
            t = pool.tile(shape, dt or f32, tag=tag)
            nc.sync.dma_start(t[:, :], dram[:, :])
            return t

        qT = ld(cpool, qT_d, [128, 2 * BPC], "qT")
        waT = ld(cpool, waT_d, [128, 512], "waT")
        uaT = ld(cpool, uaT_d, [128, 512], "uaT", f32r)
        vaT = ld(cpool, vaT_d, [128, 2], "vaT")
        abias = ld(cpool, abias_d, [128, 2 * BPC], "abias")
        ident = ld(cpool, ident_d, [128, 128], "ident")
        bcneg = ld(cpool, bcneg_d, [1, 128], "bcneg")
        bcpos = ld(cpool, bcpos_d, [1, 128], "bcpos")
        colones = ld(cpool, colones_d, [128, 1], "colones")

        # ---- q_proj (fp32): qb[p, hh*BPC+b] = sum_d Wa[hh*128+p, d] q[b, d] + abias
        qb = cpool.tile([128, 2 * BPC], f32)
        for hh in range(2):
            pq = pmisc.tile([128, BPC], f32, tag="m")
            for dh in range(2):
                j = dh * 2 + hh
                nc.tensor.matmul(
                    pq[:, :],
                    waT[:, j * 128:(j + 1) * 128],
                    qT[:, dh * BPC:(dh + 1) * BPC],
                    start=(dh == 0), stop=(dh == 1),
                )
            nc.vector.tensor_add(
                qb[:, hh * BPC:(hh + 1) * BPC], pq[:, :],
                abias[:, hh * BPC:(hh + 1) * BPC],
            )

        for b in range(BPC):
            kb = keys_d[b]  # [S, HE]
            ktiles = []
            pscT_b = pscT.tile([128, NTAU], f32)
            for k in range(NBLK):
                kt = kres.tile([128, SB * HE // 128], f32r, tag="kres")
                src = kb[k * SB:(k + 1) * SB, :].rearrange(
                    "(t p) e -> p t e", p=128)
                nc.sync.dma_start(
                    kt[:, :].rearrange("p (t e) -> p t e", t=4), src)
                ktiles.append(kt)

                # transpose the 4x2 [128,128] sub-blocks -> keysT [e,s]
                kTs = []
                for eh in range(2):
                    ptr = ptp.tile([128, SB], f32, tag="ptr")
                    for t in range(4):
                        nc.tensor.transpose(
                            ptr[:, t * 128:(t + 1) * 128],
                            kt[:, t * 256 + eh * 128:
                               t * 256 + (eh + 1) * 128].bitcast(f32),
                            ident[:, :],
                        )
                    kT = ktp.tile([128, SB], f32r, tag="kT")
                    nc.vector.tensor_copy(kT[:, :], ptr[:, :])
                    kTs.append(kT)

                ths = []
                for hh in range(2):
                    pk = pkp.tile([128, SB], f32, tag="pk")
                    for eh in range(2):
                        j = eh * 2 + hh
                        nc.tensor.matmul(
                            pk[:, :],
                            uaT[:, j * 128:(j + 1) * 128],
                            kTs[eh][:, :],
                            start=(eh == 0), stop=(eh == 1),
                        )
                    th = thp.tile([128, SB], f32, tag="th")
                    nc.scalar.activation(
                        th[:, :], pk[:, :], AF.Tanh,
                        bias=qb[:, hh * BPC + b: hh * BPC + b + 1], scale=1.0,
                    )
                    ths.append(th)
                # scores columns: psum_scT[:, tau] = sum_hh tanh_blk.T @ Va_hh
                # (the two matmuls of each tau's accumulation group must be
                # adjacent: start=True clears has_written bank-wide)
                for t in range(4):
                    tau = k * 4 + t
                    for hh in range(2):
                        nc.tensor.matmul(
                            pscT_b[:, tau:tau + 1],
                            ths[hh][:, t * 128:(t + 1) * 128],
                            vaT[:, hh:hh + 1],
                            start=(hh == 0), stop=(hh == 1),
                            skip_group_check=True,
                        )

            # ---- softmax over the [128, NTAU] column-layout scores
            pmax = smp.tile([128, 1], f32, tag="pmax")
            nc.vector.reduce_max(pmax[:, :], pscT_b[:, :], axis=AX.X)
            prow = pmisc.tile([1, 128], f32, tag="m")
            nc.tensor.transpose(prow[:1, :128], pmax[:, :1], ident[:, :])
            gmax = smp.tile([1, 1], f32, tag="gmax")
            nc.vector.reduce_max(gmax[:1, :1], prow[:1, :128], axis=AX.X)
            pnm = pmisc.tile([128, 1], f32, tag="m")
            nc.tensor.matmul(pnm[:, :], bcneg[:1, :], gmax[:1, :1],
                             start=True, stop=True)
            nbias = smp.tile([128, 1], f32, tag="nbias")
            nc.vector.tensor_copy(nbias[:, :], pnm[:, :])
            probs = smp.tile([128, NTAU], f32, tag="probs")
            psums = smp.tile([128, 1], f32, tag="psums")
            nc.scalar.activation(
                probs[:, :], pscT_b[:, :], AF.Exp,
                bias=nbias[:, :1], scale=1.0, accum_out=psums[:, :1],
            )
            pl = pmisc.tile([1, 1], f32, tag="m")
            nc.tensor.matmul(pl[:1, :1], colones[:, :1], psums[:, :1],
                             start=True, stop=True)
            rinv = smp.tile([1, 1], f32, tag="rinv")
            nc.vector.reciprocal(rinv[:1, :1], pl[:1, :1])
            pri = pmisc.tile([128, 1], f32, tag="m")
            nc.tensor.matmul(pri[:, :], bcpos[:1, :], rinv[:1, :1],
                             start=True, stop=True)
            rfac = smp.tile([128, 1], f32, tag="rfac")
            nc.vector.tensor_copy(rfac[:, :], pri[:, :])
            wcT = wcp.tile([128, NTAU], f32r, tag="wcT")
            nc.vector.tensor_scalar_mul(wcT[:, :], probs[:, :], rfac[:, :1])

            # ---- context: accumulate over resident natural-layout keys tiles
            pc = pctx.tile([1, HE], f32, tag="pc")
            for tau in range(NTAU):
                k, t = tau // 4, tau % 4
                nc.tensor.matmul(
                    pc[:1, :],
                    wcT[:, tau:tau + 1],
                    ktiles[k][:, t * 256:(t + 1) * 256],
                    start=(tau == 0), stop=(tau == NTAU - 1),
                )
            crow = orow.tile([1, HE], f32, tag="crow")
            nc.vector.tensor_copy(crow[:1, :], pc[:1, :])
            nc.sync.dma_start(octx_d[b:b + 1, :], crow[:1, :])

            # ---- weights out: transpose probs [128, NTAU] -> [NTAU, 128],
            # normalize in row layout (keeps full f32 for the output)
            pwt = pmisc.tile([NTAU, 128], f32, tag="m")
            nc.tensor.transpose(pwt[:, :], probs[:, :], ident[:, :])
            wrow = orow.tile([NTAU, 128], f32, tag="wrow")
            nc.vector.tensor_scalar_mul(wrow[:, :], pwt[:, :], rfac[:NTAU, :1])
            nc.sync.dma_start(
                ow_d[b, :].rearrange("(t p) -> t p", p=128), wrow[:, :])

    nc.compile()
    return nc


def _get_program():
    global _PROGRAM
    if _PROGRAM is None:
        _PROGRAM = _build_program()
    return _PROGRAM


def kernel(query, keys, Wa_w, Wa_b, Ua_w, Ua_b, Va_w, Va_b):
    from concourse import bass_utils

    query = np.asarray(query, dtype=np.float32)
    keys = np.asarray(keys, dtype=np.float32)
    Wa_w = np.asarray(Wa_w, dtype=np.float32)
    Wa_b = np.asarray(Wa_b, dtype=np.float32)
    Ua_w = np.asarray(Ua_w, dtype=np.float32)
    Ua_b = np.asarray(Ua_b, dtype=np.float32)
    Va_w = np.asarray(Va_w, dtype=np.float32)

    nc = _get_program()

    # host-side layout prep (pure reshapes/transposes of small weights)
    waT = np.zeros((128, 512), np.float32)
    uaT = np.zeros((128, 512), np.float32)
    for dh in range(2):
        for hh in range(2):
            j = dh * 2 + hh
            waT[:, j * 128:(j + 1) * 128] = \
                Wa_w[hh * 128:(hh + 1) * 128, dh * 128:(dh + 1) * 128].T
            uaT[:, j * 128:(j + 1) * 128] = \
                Ua_w[hh * 128:(hh + 1) * 128, dh * 128:(dh + 1) * 128].T
    vaT = Va_w[0].reshape(2, 128).T.copy()          # [128, 2]
    bias_h = (Wa_b + Ua_b).reshape(2, 128).T        # [128, 2]
    abias = np.ascontiguousarray(np.concatenate(
        [np.tile(bias_h[:, hh:hh + 1], (1, BPC)) for hh in range(2)], axis=1))
    ident = np.eye(128, dtype=np.float32)
    bcneg = np.full((1, 128), -1.0, np.float32)
    bcpos = np.full((1, 128), 1.0, np.float32)
    colones = np.ones((128, 1), np.float32)

    in_maps = []
    for c in range(NCORES):
        bsl = slice(c * BPC, (c + 1) * BPC)
        qc = query[bsl, 0, :]                        # [BPC, 256]
        # qT2[p, dh*BPC + b] = q[b, dh*128 + p]
        qT2 = np.zeros((128, 2 * BPC), np.float32)
        for dh in range(2):
            qT2[:, dh * BPC:(dh + 1) * BPC] = qc[:, dh * 128:(dh + 1) * 128].T
        in_maps.append({
            "keys": np.ascontiguousarray(keys[bsl]),
            "qT": qT2,
            "waT": waT, "uaT": uaT, "vaT": vaT, "abias": abias,
            "ident": ident, "bcneg": bcneg, "bcpos": bcpos,
            "colones": colones,
        })

    global LAST_RESULT
    kw = {}
    if PROFILE:
        kw = dict(trace=True, tmpdir="/root/problem/trace_out")
    res = bass_utils.run_bass_kernel_spmd(nc, in_maps, list(range(NCORES)), **kw)
    LAST_RESULT = res
    ctx = np.zeros((B, 1, HE), np.float32)
    wts = np.zeros((B, 1, S), np.float32)
    for c in range(NCORES):
        ctx[c * BPC:(c + 1) * BPC, 0, :] = res.results[c]["octx"]
        wts[c * BPC:(c + 1) * BPC, 0, :] = res.results[c]["ow"]
    return (ctx, wts)


# revision 22
# speedup vs baseline: 1.3701x; 1.3701x over previous
"""Bahdanau attention Trainium2 kernel.

Problem: B=32, S=8192, HE=HD=256.
  q_proj = query @ Wa^T + Wa_b                  [B,1,256]
  k_proj = keys @ Ua^T + Ua_b                   [B,S,256]
  scores = tanh(q_proj + k_proj) @ Va + Va_b    [B,S]   (Va_b dropped: softmax shift-invariant)
  weights = softmax(scores)                     [B,1,S]
  context = weights @ keys                      [B,1,256]

Sharding: pure data parallel over batch, 4 batches per core x 8 cores.
Per core, per batch: keys [8192,256] stays resident in SBUF (8MB); streamed in
512-row blocks through PE-transpose -> Ua matmul (float32r) -> tanh(+qbias) on
ACT -> Va matvec producing scores in column layout [128,64]; softmax with
cross-partition reductions done via tiny PE matmuls; context accumulated over
the resident natural-layout keys tiles.
"""

import sys
import numpy as np
import ml_dtypes

for p in ("/opt/trn_rl_repo", "/root/.axon_site/_ro/trn_rl_repo"):
    if p not in sys.path:
        sys.path.append(p)

B, S, HE, HD = 32, 8192, 256, 256
NCORES = 8
BPC = B // NCORES          # batches per core = 4
SB = 512                   # s-block streamed per iteration
NBLK = S // SB             # 16 blocks per batch
NTAU = S // 128            # 64 s-subtiles of 128 per batch

_PROGRAM = None
PROFILE = False          # set by test.py to collect an NTFF trace
LAST_RESULT = None


def _build_program():
    import concourse.bass as bass
    import concourse.bacc as bacc
    import concourse.mybir as mybir
    from concourse import tile
    from contextlib import ExitStack

    f32 = mybir.dt.float32
    f32r = mybir.dt.float32r
    bf16 = mybir.dt.bfloat16
    AF = mybir.ActivationFunctionType
    AX = mybir.AxisListType

    nc = bacc.Bacc(trn_type="TRN2")

    keys_d = nc.dram_tensor("keys", [BPC, S, HE], f32, kind="ExternalInput")
    qT_d = nc.dram_tensor("qT", [128, 2 * BPC], f32, kind="ExternalInput")
    waT_d = nc.dram_tensor("waT", [128, 512], f32, kind="ExternalInput")
    uaT_d = nc.dram_tensor("uaT", [128, 512], bf16, kind="ExternalInput")
    vaT_d = nc.dram_tensor("vaT", [128, 2], bf16, kind="ExternalInput")
    abias_d = nc.dram_tensor("abias", [128, 2 * BPC], f32, kind="ExternalInput")
    ident_d = nc.dram_tensor("ident", [128, 128], f32, kind="ExternalInput")
    identb_d = nc.dram_tensor("identb", [128, 128], bf16, kind="ExternalInput")
    bcneg_d = nc.dram_tensor("bcneg", [1, 128], f32, kind="ExternalInput")
    bcpos_d = nc.dram_tensor("bcpos", [1, 128], f32, kind="ExternalInput")
    colones_d = nc.dram_tensor("colones", [128, 1], f32, kind="ExternalInput")

    octx_d = nc.dram_tensor("octx", [BPC, HE], f32, kind="ExternalOutput")
    ow_d = nc.dram_tensor("ow", [BPC, S], f32, kind="ExternalOutput")

    with tile.TileContext(nc) as tc, ExitStack() as ctx:
        cpool = ctx.enter_context(tc.tile_pool(name="consts", bufs=1))
        kstream = ctx.enter_context(tc.tile_pool(name="kstream", bufs=6))
        kres = ctx.enter_context(tc.tile_pool(name="kres", bufs=2 * NBLK))
        ktp = ctx.enter_context(tc.tile_pool(name="ktp", bufs=4))
        thp = ctx.enter_context(tc.tile_pool(name="thp", bufs=4))
        smp = ctx.enter_context(tc.tile_pool(name="smp", bufs=2))
        wcp = ctx.enter_context(tc.tile_pool(name="wcp", bufs=2))
        orow = ctx.enter_context(tc.tile_pool(name="orow", bufs=2))

        ptp = ctx.enter_context(tc.tile_pool(name="ptp", bufs=2, space="PSUM"))
        pkp = ctx.enter_context(tc.tile_pool(name="pkp", bufs=2, space="PSUM"))
        pscT = ctx.enter_context(tc.tile_pool(name="pscT", bufs=2, space="PSUM"))
        pctx = ctx.enter_context(tc.tile_pool(name="pctx", bufs=1, space="PSUM"))
        pmisc = ctx.enter_context(tc.tile_pool(name="pmisc", bufs=1, space="PSUM"))

        def ld(pool, dram, shape, tag, dt=None):
            t = pool.tile(shape, dt or f32, tag=tag)
            nc.sync.dma_start(t[:, :], dram[:, :])
            return t

        qT = ld(cpool, qT_d, [128, 2 * BPC], "qT")
        waT = ld(cpool, waT_d, [128, 512], "waT")
        uaT = ld(cpool, uaT_d, [128, 512], "uaT", bf16)
        vaT = ld(cpool, vaT_d, [128, 2], "vaT", bf16)
        abias = ld(cpool, abias_d, [128, 2 * BPC], "abias")
        ident = ld(cpool, ident_d, [128, 128], "ident")
        identb = ld(cpool, identb_d, [128, 128], "identb", bf16)
        bcneg = ld(cpool, bcneg_d, [1, 128], "bcneg")
        bcpos = ld(cpool, bcpos_d, [1, 128], "bcpos")
        colones = ld(cpool, colones_d, [128, 1], "colones")

        # ---- q_proj (fp32): qb[p, hh*BPC+b] = sum_d Wa[hh*128+p, d] q[b, d] + abias
        qb = cpool.tile([128, 2 * BPC], f32)
        for hh in range(2):
            pq = pmisc.tile([128, BPC], f32, tag="m")
            for dh in range(2):
                j = dh * 2 + hh
                nc.tensor.matmul(
                    pq[:, :],
                    waT[:, j * 128:(j + 1) * 128],
                    qT[:, dh * BPC:(dh + 1) * BPC],
                    start=(dh == 0), stop=(dh == 1),
                )
            nc.vector.tensor_add(
                qb[:, hh * BPC:(hh + 1) * BPC], pq[:, :],
                abias[:, hh * BPC:(hh + 1) * BPC],
            )

        for b in range(BPC):
            kb = keys_d[b]  # [S, HE]
            ktiles = []
            pscT_b = pscT.tile([128, NTAU], f32)
            for k in range(NBLK):
                kf = kstream.tile([128, SB * HE // 128], f32, tag="ks")
                src = kb[k * SB:(k + 1) * SB, :].rearrange(
                    "(t p) e -> p t e", p=128)
                nc.sync.dma_start(
                    kf[:, :].rearrange("p (t e) -> p t e", t=4), src)
                kt = kres.tile([128, SB * HE // 128], bf16, tag="kres")
                nc.gpsimd.tensor_copy(kt[:, :], kf[:, :])
                ktiles.append(kt)

                # transpose the 4x2 [128,128] sub-blocks -> keysT [e,s]
                kTs = []
                for eh in range(2):
                    ptr = ptp.tile([128, SB], bf16, tag="ptr")
                    for t in range(4):
                        nc.tensor.transpose(
                            ptr[:, t * 128:(t + 1) * 128],
                            kt[:, t * 256 + eh * 128:
                               t * 256 + (eh + 1) * 128],
                            identb[:, :],
                        )
                    kT = ktp.tile([128, SB], bf16, tag="kT")
                    nc.vector.tensor_copy(kT[:, :], ptr[:, :])
                    kTs.append(kT)

                ths = []
                for hh in range(2):
                    pk = pkp.tile([128, SB], f32, tag="pk")
                    for eh in range(2):
                        j = eh * 2 + hh
                        nc.tensor.matmul(
                            pk[:, :],
                            uaT[:, j * 128:(j + 1) * 128],
                            kTs[eh][:, :],
                            start=(eh == 0), stop=(eh == 1),
                        )
                    th = thp.tile([128, SB], bf16, tag="th")
                    nc.scalar.activation(
                        th[:, :], pk[:, :], AF.Tanh,
                        bias=qb[:, hh * BPC + b: hh * BPC + b + 1], scale=1.0,
                    )
                    ths.append(th)
                # scores columns: psum_scT[:, tau] = sum_hh tanh_blk.T @ Va_hh
                # (the two matmuls of each tau's accumulation group must be
                # adjacent: start=True clears has_written bank-wide)
                for t in range(4):
                    tau = k * 4 + t
                    for hh in range(2):
                        nc.tensor.matmul(
                            pscT_b[:, tau:tau + 1],
                            ths[hh][:, t * 128:(t + 1) * 128],
                            vaT[:, hh:hh + 1],
                            start=(hh == 0), stop=(hh == 1),
                            skip_group_check=True,
                        )

            # ---- softmax over the [128, NTAU] column-layout scores
            pmax = smp.tile([128, 1], f32, tag="pmax")
            nc.vector.reduce_max(pmax[:, :], pscT_b[:, :], axis=AX.X)
            prow = pmisc.tile([1, 128], f32, tag="m")
            nc.tensor.transpose(prow[:1, :128], pmax[:, :1], ident[:, :])
            gmax = smp.tile([1, 1], f32, tag="gmax")
            nc.vector.reduce_max(gmax[:1, :1], prow[:1, :128], axis=AX.X)
            pnm = pmisc.tile([128, 1], f32, tag="m")
            nc.tensor.matmul(pnm[:, :], bcneg[:1, :], gmax[:1, :1],
                             start=True, stop=True)
            nbias = smp.tile([128, 1], f32, tag="nbias")
            nc.vector.tensor_copy(nbias[:, :], pnm[:, :])
            probs = smp.tile([128, NTAU], f32, tag="probs")
            psums = smp.tile([128, 1], f32, tag="psums")
            nc.scalar.activation(
                probs[:, :], pscT_b[:, :], AF.Exp,
                bias=nbias[:, :1], scale=1.0, accum_out=psums[:, :1],
            )
            pl = pmisc.tile([1, 1], f32, tag="m")
            nc.tensor.matmul(pl[:1, :1], colones[:, :1], psums[:, :1],
                             start=True, stop=True)
            rinv = smp.tile([1, 1], f32, tag="rinv")
            nc.vector.reciprocal(rinv[:1, :1], pl[:1, :1])
            pri = pmisc.tile([128, 1], f32, tag="m")
            nc.tensor.matmul(pri[:, :], bcpos[:1, :], rinv[:1, :1],
                             start=True, stop=True)
            rfac = smp.tile([128, 1], f32, tag="rfac")
            nc.vector.tensor_copy(rfac[:, :], pri[:, :])
            wcT = wcp.tile([128, NTAU], bf16, tag="wcT")
            nc.vector.tensor_scalar_mul(wcT[:, :], probs[:, :], rfac[:, :1])

            # ---- context: accumulate over resident natural-layout keys tiles
            pc = pctx.tile([1, HE], f32, tag="pc")
            for tau in range(NTAU):
                k, t = tau // 4, tau % 4
                nc.tensor.matmul(
                    pc[:1, :],
                    wcT[:, tau:tau + 1],
                    ktiles[k][:, t * 256:(t + 1) * 256],
                    start=(tau == 0), stop=(tau == NTAU - 1),
                )
            crow = orow.tile([1, HE], f32, tag="crow")
            nc.vector.tensor_copy(crow[:1, :], pc[:1, :])
            nc.sync.dma_start(octx_d[b:b + 1, :], crow[:1, :])

            # ---- weights out: transpose probs [128, NTAU] -> [NTAU, 128],
            # normalize in row layout (keeps full f32 for the output)
            pwt = pmisc.tile([NTAU, 128], f32, tag="m")
            nc.tensor.transpose(pwt[:, :], probs[:, :], ident[:, :])
            wrow = orow.tile([NTAU, 128], f32, tag="wrow")
            nc.vector.tensor_scalar_mul(wrow[:, :], pwt[:, :], rfac[:NTAU, :1])
            nc.sync.dma_start(
                ow_d[b, :].rearrange("(t p) -> t p", p=128), wrow[:, :])

    nc.compile()
    return nc


def _get_program():
    global _PROGRAM
    if _PROGRAM is None:
        _PROGRAM = _build_program()
    return _PROGRAM


def kernel(query, keys, Wa_w, Wa_b, Ua_w, Ua_b, Va_w, Va_b):
    from concourse import bass_utils

    query = np.asarray(query, dtype=np.float32)
    keys = np.asarray(keys, dtype=np.float32)
    Wa_w = np.asarray(Wa_w, dtype=np.float32)
    Wa_b = np.asarray(Wa_b, dtype=np.float32)
    Ua_w = np.asarray(Ua_w, dtype=np.float32)
    Ua_b = np.asarray(Ua_b, dtype=np.float32)
    Va_w = np.asarray(Va_w, dtype=np.float32)

    nc = _get_program()

    # host-side layout prep (pure reshapes/transposes of small weights)
    waT = np.zeros((128, 512), np.float32)
    uaT = np.zeros((128, 512), np.float32)
    for dh in range(2):
        for hh in range(2):
            j = dh * 2 + hh
            waT[:, j * 128:(j + 1) * 128] = \
                Wa_w[hh * 128:(hh + 1) * 128, dh * 128:(dh + 1) * 128].T
            uaT[:, j * 128:(j + 1) * 128] = \
                Ua_w[hh * 128:(hh + 1) * 128, dh * 128:(dh + 1) * 128].T
    vaT = Va_w[0].reshape(2, 128).T.copy()          # [128, 2]
    bias_h = (Wa_b + Ua_b).reshape(2, 128).T        # [128, 2]
    abias = np.ascontiguousarray(np.concatenate(
        [np.tile(bias_h[:, hh:hh + 1], (1, BPC)) for hh in range(2)], axis=1))
    ident = np.eye(128, dtype=np.float32)
    identb = np.eye(128, dtype=np.float32).astype(ml_dtypes.bfloat16)
    bcneg = np.full((1, 128), -1.0, np.float32)
    bcpos = np.full((1, 128), 1.0, np.float32)
    colones = np.ones((128, 1), np.float32)

    in_maps = []
    for c in range(NCORES):
        bsl = slice(c * BPC, (c + 1) * BPC)
        qc = query[bsl, 0, :]                        # [BPC, 256]
        # qT2[p, dh*BPC + b] = q[b, dh*128 + p]
        qT2 = np.zeros((128, 2 * BPC), np.float32)
        for dh in range(2):
            qT2[:, dh * BPC:(dh + 1) * BPC] = qc[:, dh * 128:(dh + 1) * 128].T
        in_maps.append({
            "keys": np.ascontiguousarray(keys[bsl]),
            "qT": qT2,
            "waT": waT, "uaT": uaT.astype(ml_dtypes.bfloat16),
            "vaT": vaT.astype(ml_dtypes.bfloat16), "abias": abias,
            "ident": ident, "identb": identb,
            "bcneg": bcneg, "bcpos": bcpos,
            "colones": colones,
        })

    global LAST_RESULT
    kw = {}
    if PROFILE:
        kw = dict(trace=True, tmpdir="/root/problem/trace_out")
    res = bass_utils.run_bass_kernel_spmd(nc, in_maps, list(range(NCORES)), **kw)
    LAST_RESULT = res
    ctx = np.zeros((B, 1, HE), np.float32)
    wts = np.zeros((B, 1, S), np.float32)
    for c in range(NCORES):
        ctx[c * BPC:(c + 1) * BPC, 0, :] = res.results[c]["octx"]
        wts[c * BPC:(c + 1) * BPC, 0, :] = res.results[c]["ow"]
    return (ctx, wts)


# revision 24
# speedup vs baseline: 2.2926x; 1.6733x over previous
"""Bahdanau attention Trainium2 kernel.

Problem: B=32, S=8192, HE=HD=256.
  q_proj = query @ Wa^T + Wa_b                  [B,1,256]
  k_proj = keys @ Ua^T + Ua_b                   [B,S,256]
  scores = tanh(q_proj + k_proj) @ Va + Va_b    [B,S]   (Va_b dropped: softmax shift-invariant)
  weights = softmax(scores)                     [B,1,S]
  context = weights @ keys                      [B,1,256]

Sharding: pure data parallel over batch, 4 batches per core x 8 cores.
Per core, per batch: keys [8192,256] stays resident in SBUF (8MB); streamed in
512-row blocks through PE-transpose -> Ua matmul (float32r) -> tanh(+qbias) on
ACT -> Va matvec producing scores in column layout [128,64]; softmax with
cross-partition reductions done via tiny PE matmuls; context accumulated over
the resident natural-layout keys tiles.
"""

import sys
import numpy as np
import ml_dtypes

for p in ("/opt/trn_rl_repo", "/root/.axon_site/_ro/trn_rl_repo"):
    if p not in sys.path:
        sys.path.append(p)

B, S, HE, HD = 32, 8192, 256, 256
NCORES = 8
BPC = B // NCORES          # batches per core = 4
SB = 512                   # s-block streamed per iteration
NBLK = S // SB             # 16 blocks per batch
NTAU = S // 128            # 64 s-subtiles of 128 per batch

_PROGRAM = None
PROFILE = False          # set by test.py to collect an NTFF trace
LAST_RESULT = None


def _build_program():
    import concourse.bass as bass
    import concourse.bacc as bacc
    import concourse.mybir as mybir
    from concourse import tile
    from contextlib import ExitStack

    f32 = mybir.dt.float32
    f32r = mybir.dt.float32r
    bf16 = mybir.dt.bfloat16
    AF = mybir.ActivationFunctionType
    AX = mybir.AxisListType

    nc = bacc.Bacc(trn_type="TRN2", num_swdge_queues=4)

    keys_d = nc.dram_tensor("keys", [BPC, S, HE], f32, kind="ExternalInput")
    qT_d = nc.dram_tensor("qT", [128, 2 * BPC], f32, kind="ExternalInput")
    waT_d = nc.dram_tensor("waT", [128, 512], f32, kind="ExternalInput")
    uaT_d = nc.dram_tensor("uaT", [128, 512], bf16, kind="ExternalInput")
    vaT_d = nc.dram_tensor("vaT", [128, 2], bf16, kind="ExternalInput")
    abias_d = nc.dram_tensor("abias", [128, 2 * BPC], f32, kind="ExternalInput")
    ident_d = nc.dram_tensor("ident", [128, 128], f32, kind="ExternalInput")
    identb_d = nc.dram_tensor("identb", [128, 128], bf16, kind="ExternalInput")
    bcneg_d = nc.dram_tensor("bcneg", [1, 128], f32, kind="ExternalInput")
    bcpos_d = nc.dram_tensor("bcpos", [1, 128], f32, kind="ExternalInput")
    colones_d = nc.dram_tensor("colones", [128, 1], f32, kind="ExternalInput")

    octx_d = nc.dram_tensor("octx", [BPC, HE], f32, kind="ExternalOutput")
    ow_d = nc.dram_tensor("ow", [BPC, S], f32, kind="ExternalOutput")

    with tile.TileContext(nc) as tc, ExitStack() as ctx:
        cpool = ctx.enter_context(tc.tile_pool(name="consts", bufs=1))
        kres = ctx.enter_context(tc.tile_pool(name="kres", bufs=2 * NBLK))
        ktp = ctx.enter_context(tc.tile_pool(name="ktp", bufs=4))
        thp = ctx.enter_context(tc.tile_pool(name="thp", bufs=4))
        smp = ctx.enter_context(tc.tile_pool(name="smp", bufs=2))
        wcp = ctx.enter_context(tc.tile_pool(name="wcp", bufs=2))
        orow = ctx.enter_context(tc.tile_pool(name="orow", bufs=2))

        ptp = ctx.enter_context(tc.tile_pool(name="ptp", bufs=2, space="PSUM"))
        pkp = ctx.enter_context(tc.tile_pool(name="pkp", bufs=2, space="PSUM"))
        pscT = ctx.enter_context(tc.tile_pool(name="pscT", bufs=2, space="PSUM"))
        pctx = ctx.enter_context(tc.tile_pool(name="pctx", bufs=1, space="PSUM"))
        pmisc = ctx.enter_context(tc.tile_pool(name="pmisc", bufs=1, space="PSUM"))

        def ld(pool, dram, shape, tag, dt=None):
            t = pool.tile(shape, dt or f32, tag=tag)
            nc.sync.dma_start(t[:, :], dram[:, :])
            return t

        qT = ld(cpool, qT_d, [128, 2 * BPC], "qT")
        waT = ld(cpool, waT_d, [128, 512], "waT")
        uaT = ld(cpool, uaT_d, [128, 512], "uaT", bf16)
        vaT = ld(cpool, vaT_d, [128, 2], "vaT", bf16)
        abias = ld(cpool, abias_d, [128, 2 * BPC], "abias")
        ident = ld(cpool, ident_d, [128, 128], "ident")
        identb = ld(cpool, identb_d, [128, 128], "identb", bf16)
        bcneg = ld(cpool, bcneg_d, [1, 128], "bcneg")
        bcpos = ld(cpool, bcpos_d, [1, 128], "bcpos")
        colones = ld(cpool, colones_d, [128, 1], "colones")

        # ---- q_proj (fp32): qb[p, hh*BPC+b] = sum_d Wa[hh*128+p, d] q[b, d] + abias
        qb = cpool.tile([128, 2 * BPC], f32)
        for hh in range(2):
            pq = pmisc.tile([128, BPC], f32, tag="m")
            for dh in range(2):
                j = dh * 2 + hh
                nc.tensor.matmul(
                    pq[:, :],
                    waT[:, j * 128:(j + 1) * 128],
                    qT[:, dh * BPC:(dh + 1) * BPC],
                    start=(dh == 0), stop=(dh == 1),
                )
            nc.vector.tensor_add(
                qb[:, hh * BPC:(hh + 1) * BPC], pq[:, :],
                abias[:, hh * BPC:(hh + 1) * BPC],
            )

        for b in range(BPC):
            kb = keys_d[b]  # [S, HE]
            ktiles = []
            pscT_b = pscT.tile([128, NTAU], f32)
            for k in range(NBLK):
                src = kb[k * SB:(k + 1) * SB, :].rearrange(
                    "(t p) e -> p t e", p=128)
                kt = kres.tile([128, SB * HE // 128], bf16, tag="kres")
                nc.gpsimd.dma_start(
                    kt[:, :].rearrange("p (t e) -> p t e", t=4), src)
                ktiles.append(kt)

                # transpose the 4x2 [128,128] sub-blocks -> keysT [e,s]
                kTs = []
                for eh in range(2):
                    ptr = ptp.tile([128, SB], bf16, tag="ptr")
                    for t in range(4):
                        nc.tensor.transpose(
                            ptr[:, t * 128:(t + 1) * 128],
                            kt[:, t * 256 + eh * 128:
                               t * 256 + (eh + 1) * 128],
                            identb[:, :],
                        )
                    kT = ktp.tile([128, SB], bf16, tag="kT")
                    nc.vector.tensor_copy(kT[:, :], ptr[:, :])
                    kTs.append(kT)

                ths = []
                for hh in range(2):
                    pk = pkp.tile([128, SB], f32, tag="pk")
                    for eh in range(2):
                        j = eh * 2 + hh
                        nc.tensor.matmul(
                            pk[:, :],
                            uaT[:, j * 128:(j + 1) * 128],
                            kTs[eh][:, :],
                            start=(eh == 0), stop=(eh == 1),
                        )
                    th = thp.tile([128, SB], bf16, tag="th")
                    nc.scalar.activation(
                        th[:, :], pk[:, :], AF.Tanh,
                        bias=qb[:, hh * BPC + b: hh * BPC + b + 1], scale=1.0,
                    )
                    ths.append(th)
                # scores columns: psum_scT[:, tau] = sum_hh tanh_blk.T @ Va_hh
                # (the two matmuls of each tau's accumulation group must be
                # adjacent: start=True clears has_written bank-wide)
                for t in range(4):
                    tau = k * 4 + t
                    for hh in range(2):
                        nc.tensor.matmul(
                            pscT_b[:, tau:tau + 1],
                            ths[hh][:, t * 128:(t + 1) * 128],
                            vaT[:, hh:hh + 1],
                            start=(hh == 0), stop=(hh == 1),
                            skip_group_check=True,
                        )

            # ---- softmax over the [128, NTAU] column-layout scores
            pmax = smp.tile([128, 1], f32, tag="pmax")
            nc.vector.reduce_max(pmax[:, :], pscT_b[:, :], axis=AX.X)
            prow = pmisc.tile([1, 128], f32, tag="m")
            nc.tensor.transpose(prow[:1, :128], pmax[:, :1], ident[:, :])
            gmax = smp.tile([1, 1], f32, tag="gmax")
            nc.vector.reduce_max(gmax[:1, :1], prow[:1, :128], axis=AX.X)
            pnm = pmisc.tile([128, 1], f32, tag="m")
            nc.tensor.matmul(pnm[:, :], bcneg[:1, :], gmax[:1, :1],
                             start=True, stop=True)
            nbias = smp.tile([128, 1], f32, tag="nbias")
            nc.vector.tensor_copy(nbias[:, :], pnm[:, :])
            probs = smp.tile([128, NTAU], f32, tag="probs")
            psums = smp.tile([128, 1], f32, tag="psums")
            nc.scalar.activation(
                probs[:, :], pscT_b[:, :], AF.Exp,
                bias=nbias[:, :1], scale=1.0, accum_out=psums[:, :1],
            )
            pl = pmisc.tile([1, 1], f32, tag="m")
            nc.tensor.matmul(pl[:1, :1], colones[:, :1], psums[:, :1],
                             start=True, stop=True)
            rinv = smp.tile([1, 1], f32, tag="rinv")
            nc.vector.reciprocal(rinv[:1, :1], pl[:1, :1])
            pri = pmisc.tile([128, 1], f32, tag="m")
            nc.tensor.matmul(pri[:, :], bcpos[:1, :], rinv[:1, :1],
                             start=True, stop=True)
            rfac = smp.tile([128, 1], f32, tag="rfac")
            nc.vector.tensor_copy(rfac[:, :], pri[:, :])
            wcT = wcp.tile([128, NTAU], bf16, tag="wcT")
            nc.vector.tensor_scalar_mul(wcT[:, :], probs[:, :], rfac[:, :1])

            # ---- context: accumulate over resident natural-layout keys tiles
            pc = pctx.tile([1, HE], f32, tag="pc")
            for tau in range(NTAU):
                k, t = tau // 4, tau % 4
                nc.tensor.matmul(
                    pc[:1, :],
                    wcT[:, tau:tau + 1],
                    ktiles[k][:, t * 256:(t + 1) * 256],
                    start=(tau == 0), stop=(tau == NTAU - 1),
                )
            crow = orow.tile([1, HE], f32, tag="crow")
            nc.vector.tensor_copy(crow[:1, :], pc[:1, :])
            nc.sync.dma_start(octx_d[b:b + 1, :], crow[:1, :])

            # ---- weights out: transpose probs [128, NTAU] -> [NTAU, 128],
            # normalize in row layout (keeps full f32 for the output)
            pwt = pmisc.tile([NTAU, 128], f32, tag="m")
            nc.tensor.transpose(pwt[:, :], probs[:, :], ident[:, :])
            wrow = orow.tile([NTAU, 128], f32, tag="wrow")
            nc.vector.tensor_scalar_mul(wrow[:, :], pwt[:, :], rfac[:NTAU, :1])
            nc.sync.dma_start(
                ow_d[b, :].rearrange("(t p) -> t p", p=128), wrow[:, :])

    nc.compile()
    return nc


def _get_program():
    global _PROGRAM
    if _PROGRAM is None:
        _PROGRAM = _build_program()
    return _PROGRAM


def kernel(query, keys, Wa_w, Wa_b, Ua_w, Ua_b, Va_w, Va_b):
    from concourse import bass_utils

    query = np.asarray(query, dtype=np.float32)
    keys = np.asarray(keys, dtype=np.float32)
    Wa_w = np.asarray(Wa_w, dtype=np.float32)
    Wa_b = np.asarray(Wa_b, dtype=np.float32)
    Ua_w = np.asarray(Ua_w, dtype=np.float32)
    Ua_b = np.asarray(Ua_b, dtype=np.float32)
    Va_w = np.asarray(Va_w, dtype=np.float32)

    nc = _get_program()

    # host-side layout prep (pure reshapes/transposes of small weights)
    waT = np.zeros((128, 512), np.float32)
    uaT = np.zeros((128, 512), np.float32)
    for dh in range(2):
        for hh in range(2):
            j = dh * 2 + hh
            waT[:, j * 128:(j + 1) * 128] = \
                Wa_w[hh * 128:(hh + 1) * 128, dh * 128:(dh + 1) * 128].T
            uaT[:, j * 128:(j + 1) * 128] = \
                Ua_w[hh * 128:(hh + 1) * 128, dh * 128:(dh + 1) * 128].T
    vaT = Va_w[0].reshape(2, 128).T.copy()          # [128, 2]
    bias_h = (Wa_b + Ua_b).reshape(2, 128).T        # [128, 2]
    abias = np.ascontiguousarray(np.concatenate(
        [np.tile(bias_h[:, hh:hh + 1], (1, BPC)) for hh in range(2)], axis=1))
    ident = np.eye(128, dtype=np.float32)
    identb = np.eye(128, dtype=np.float32).astype(ml_dtypes.bfloat16)
    bcneg = np.full((1, 128), -1.0, np.float32)
    bcpos = np.full((1, 128), 1.0, np.float32)
    colones = np.ones((128, 1), np.float32)

    in_maps = []
    for c in range(NCORES):
        bsl = slice(c * BPC, (c + 1) * BPC)
        qc = query[bsl, 0, :]                        # [BPC, 256]
        # qT2[p, dh*BPC + b] = q[b, dh*128 + p]
        qT2 = np.zeros((128, 2 * BPC), np.float32)
        for dh in range(2):
            qT2[:, dh * BPC:(dh + 1) * BPC] = qc[:, dh * 128:(dh + 1) * 128].T
        in_maps.append({
            "keys": np.ascontiguousarray(keys[bsl]),
            "qT": qT2,
            "waT": waT, "uaT": uaT.astype(ml_dtypes.bfloat16),
            "vaT": vaT.astype(ml_dtypes.bfloat16), "abias": abias,
            "ident": ident, "identb": identb,
            "bcneg": bcneg, "bcpos": bcpos,
            "colones": colones,
        })

    global LAST_RESULT
    kw = {}
    if PROFILE:
        kw = dict(trace=True, tmpdir="/root/problem/trace_out")
    res = bass_utils.run_bass_kernel_spmd(nc, in_maps, list(range(NCORES)), **kw)
    LAST_RESULT = res
    ctx = np.zeros((B, 1, HE), np.float32)
    wts = np.zeros((B, 1, S), np.float32)
    for c in range(NCORES):
        ctx[c * BPC:(c + 1) * BPC, 0, :] = res.results[c]["octx"]
        wts[c * BPC:(c + 1) * BPC, 0, :] = res.results[c]["ow"]
    return (ctx, wts)
